# revision 3
# baseline (speedup 1.0000x reference)
"""MoE SwiGLU expert kernel for Trainium2, 8 NeuronCores.

Problem: x[4,2048,4096] routed through K=4 active experts (of 16):
    g = x @ gate[k], u = x @ up[k], act = silu(g)*u, out = act @ down[k]
    out[b,s,k,h], all float32.

Sharding (8 cores): 4-way over tokens x 2-way over the expert hidden dim E.
  core c -> (tau = c//2: tokens [2048*tau, 2048*tau+2048),
             eps = c%2:  E-half [896*eps, 896*eps+896) of every active expert)
Each core computes a partial down-projection summed over its E-half; host
adds the two partials of each token quarter.

All matmuls run as float32r (full fp32 data, 1 cycle/row on the PE at
N=512 moving free dim) with fp32 PSUM accumulation.

Device-side layout per core (all f32):
  xT  [4096, 2048]   x^T slice (h on partitions after tiling)
  gw  [4, 4096, 896] gate weights for 4 experts, this E-half
  uw  [4, 4096, 896] up weights
  dw  [4, 896, 4096] down weights
  out [4, 4096, 2048] out^T partials per expert

Compute loop: 2 token blocks of 1024 (PSUM subtiles of 512).
x^T block [4096, 1024] stays resident in SBUF (128 KB/partition);
weights stream through small double-buffered pools.
"""
import functools
import sys

sys.path.insert(0, "/opt/trn_rl_repo")

import numpy as np

import concourse.bass as bass
import concourse.mybir as mybir
import concourse.tile as tile
from concourse import bacc
from concourse.bass_utils import run_bass_kernel_spmd

F32 = mybir.dt.float32
F32R = mybir.dt.float32r

B, S, H, E, NEXP, K = 4, 2048, 4096, 1792, 16, 4
N_CORES = 8
TOK = B * S                  # 8192 tokens
TOK_PC = TOK // 4            # 2048 tokens per core (4-way token split)
E_PC = E // 2                # 896 e-channels per core (2-way E split)
N_ET = E_PC // 128           # 7 e-tiles
N_HT = H // 128              # 32 h-tiles
TBLK = 1024                  # token block resident in SBUF
N_BLK = TOK_PC // TBLK       # 2 blocks
TSUB = 512                   # PSUM moving free dim
N_TSUB = TBLK // TSUB        # 2


def _build(n_experts=K, n_blocks=N_BLK, n_etiles=N_ET, n_htiles=N_HT):
    """Build the per-core SPMD program. Reduced sizes for dev testing."""
    nc = bacc.Bacc(
        "TRN2",
        target_bir_lowering=False,
        debug=False,
        enable_asserts=False,
        num_devices=N_CORES,
    )
    e_pc = n_etiles * 128
    h_full = n_htiles * 128
    xT = nc.dram_tensor("xT", [h_full, TOK_PC], F32R, kind="ExternalInput")
    gw = nc.dram_tensor("gw", [n_experts, h_full, e_pc], F32R, kind="ExternalInput")
    uw = nc.dram_tensor("uw", [n_experts, h_full, e_pc], F32R, kind="ExternalInput")
    dw = nc.dram_tensor("dw", [n_experts, e_pc, h_full], F32R, kind="ExternalInput")
    out = nc.dram_tensor("out", [n_experts, h_full, TOK_PC], F32, kind="ExternalOutput")

    silu = mybir.ActivationFunctionType.Silu
    hh_per_chunk = n_htiles // 2  # weight h-half chunks

    with tile.TileContext(nc) as tc:
        with (
            tc.tile_pool(name="xpool", bufs=1) as xpool,
            tc.tile_pool(name="gupool", bufs=3) as gupool,
            tc.tile_pool(name="dpool", bufs=2) as dpool,
            tc.tile_pool(name="actpool", bufs=n_etiles + 1) as actpool,
            tc.tile_pool(name="silpool", bufs=3) as silpool,
            tc.tile_pool(name="opool", bufs=2) as opool,
            tc.tile_pool(name="gups", bufs=6, space="PSUM") as gups,
            tc.tile_pool(name="ops", bufs=2, space="PSUM") as ops,
        ):
            for blk in range(n_blocks):
                t0 = blk * TBLK
                xts = xpool.tile([128, n_htiles, TBLK], F32R, tag="x", name="xts")
                nc.sync.dma_start(
                    out=xts,
                    in_=xT[:, t0 : t0 + TBLK].rearrange("(i p) t -> p i t", p=128),
                )
                for k in range(n_experts):
                    act_tiles = []
                    for j in range(n_etiles):
                        e0 = j * 128
                        psg = [
                            gups.tile([128, TSUB], F32, tag="gups", name=f"psg{s}")
                            for s in range(N_TSUB)
                        ]
                        psu = [
                            gups.tile([128, TSUB], F32, tag="gups", name=f"psu{s}")
                            for s in range(N_TSUB)
                        ]
                        for wdram, ps in ((gw, psg), (uw, psu)):
                            for m in range(2):
                                h0 = m * hh_per_chunk * 128
                                wch = gupool.tile(
                                    [128, hh_per_chunk, 128], F32R, tag="gu", name="wch"
                                )
                                nc.sync.dma_start(
                                    out=wch,
                                    in_=wdram[
                                        k, h0 : h0 + hh_per_chunk * 128, e0 : e0 + 128
                                    ].rearrange("(hh p) e -> p hh e", p=128),
                                )
                                for hh in range(hh_per_chunk):
                                    hi = m * hh_per_chunk + hh
                                    w = wch[:, hh, :]
                                    for s in range(N_TSUB):
                                        nc.tensor.matmul(
                                            ps[s],
                                            w,
                                            xts[:, hi, s * TSUB : (s + 1) * TSUB],
                                            start=(hi == 0),
                                            stop=(hi == n_htiles - 1),
                                        )
                        act_j = actpool.tile([128, TBLK], F32R, tag="act", name="act_j")
                        for s in range(N_TSUB):
                            sil = silpool.tile([128, TSUB], F32, tag="sil", name="sil")
                            nc.scalar.activation(sil, psg[s], silu)
                            nc.vector.tensor_mul(
                                act_j[:, s * TSUB : (s + 1) * TSUB], sil, psu[s]
                            )
                        act_tiles.append(act_j)
                    for i in range(n_htiles):
                        dch = dpool.tile([128, n_etiles, 128], F32R, tag="d", name="dch")
                        nc.scalar.dma_start(
                            out=dch,
                            in_=dw[k, :, i * 128 : (i + 1) * 128].rearrange(
                                "(j p) h -> p j h", p=128
                            ),
                        )
                        pso = [
                            ops.tile([128, TSUB], F32, tag="ops", name=f"pso{s}")
                            for s in range(N_TSUB)
                        ]
                        for j in range(n_etiles):
                            w = dch[:, j, :]
                            for s in range(N_TSUB):
                                nc.tensor.matmul(
                                    pso[s],
                                    w,
                                    act_tiles[j][:, s * TSUB : (s + 1) * TSUB],
                                    start=(j == 0),
                                    stop=(j == n_etiles - 1),
                                )
                        ot = opool.tile([128, TBLK], F32, tag="ot", name="ot")
                        for s in range(N_TSUB):
                            nc.vector.tensor_copy(
                                ot[:, s * TSUB : (s + 1) * TSUB], pso[s]
                            )
                        nc.gpsimd.dma_start(
                            out=out[k, i * 128 : (i + 1) * 128, t0 : t0 + TBLK],
                            in_=ot,
                        )
    nc.compile()
    return nc


@functools.cache
def _built_full():
    return _build()


def kernel(x, gate_proj, up_proj, down_proj, expert_idx):
    x = np.asarray(x)
    idx = np.asarray(expert_idx)
    gate = np.asarray(gate_proj)[idx]  # [K, H, E]
    up = np.asarray(up_proj)[idx]
    down = np.asarray(down_proj)[idx]  # [K, E, H]

    nc = _built_full()

    xf = x.reshape(TOK, H)
    in_maps = []
    for c in range(N_CORES):
        tau, eps = divmod(c, 2)
        tsl = slice(TOK_PC * tau, TOK_PC * (tau + 1))
        esl = slice(E_PC * eps, E_PC * (eps + 1))
        in_maps.append(
            {
                "xT": np.ascontiguousarray(xf[tsl].T),
                "gw": np.ascontiguousarray(gate[:, :, esl]),
                "uw": np.ascontiguousarray(up[:, :, esl]),
                "dw": np.ascontiguousarray(down[:, esl, :]),
            }
        )

    res = run_bass_kernel_spmd(nc, in_maps, core_ids=list(range(N_CORES)))

    out = np.empty((TOK, K, H), dtype=np.float32)
    for tau in range(4):
        part = res.results[2 * tau]["out"] + res.results[2 * tau + 1]["out"]
        # part: [K, H, TOK_PC] -> [TOK_PC, K, H]
        out[TOK_PC * tau : TOK_PC * (tau + 1)] = part.transpose(2, 0, 1)
    return out.reshape(B, S, K, H)


# revision 15
# speedup vs baseline: 1.1036x; 1.1036x over previous
"""MoE SwiGLU expert kernel for Trainium2, 8 NeuronCores.

Problem: x[4,2048,4096] routed through K=4 active experts (of 16):
    g = x @ gate[k], u = x @ up[k], act = silu(g)*u, out = act @ down[k]
    out[b,s,k,h], all float32.

Sharding (8 cores): 4-way over tokens x 2-way over the expert hidden dim E.
  core c -> (tau = c//2: tokens [2048*tau, 2048*tau+2048),
             eps = c%2:  E-half [896*eps, 896*eps+896) of every active expert)
Each core computes a partial down-projection summed over its E-half; host
adds the two partials of each token quarter.

All matmuls run as float32r (full fp32 data, 1 cycle/row on the PE at
N=512 moving free dim) with fp32 PSUM accumulation.

Device-side layout per core (all f32):
  xT  [4096, 2048]   x^T slice (h on partitions after tiling)
  gw  [4, 4096, 896] gate weights for 4 experts, this E-half
  uw  [4, 4096, 896] up weights
  dw  [4, 896, 4096] down weights
  out [4, 4096, 2048] out^T partials per expert

Compute loop: 2 token blocks of 1024 (PSUM subtiles of 512).
x^T block [4096, 1024] stays resident in SBUF (128 KB/partition);
weights stream through small double-buffered pools.
"""
import functools
import sys

sys.path.insert(0, "/opt/trn_rl_repo")

import numpy as np

import concourse.bass as bass
import concourse.mybir as mybir
import concourse.tile as tile
from concourse import bacc
from concourse.bass_utils import run_bass_kernel_spmd

F32 = mybir.dt.float32
F32R = mybir.dt.float32r

B, S, H, E, NEXP, K = 4, 2048, 4096, 1792, 16, 4
N_CORES = 8
TOK = B * S                  # 8192 tokens
TOK_PC = TOK // 4            # 2048 tokens per core (4-way token split)
E_PC = E // 2                # 896 e-channels per core (2-way E split)
N_ET = E_PC // 128           # 7 e-tiles
N_HT = H // 128              # 32 h-tiles
TBLK = 1024                  # token block resident in SBUF
N_BLK = TOK_PC // TBLK       # 2 blocks
TSUB = 512                   # PSUM moving free dim
N_TSUB = TBLK // TSUB        # 2


def _build(n_experts=K, n_blocks=N_BLK, n_etiles=N_ET, n_htiles=N_HT):
    """Build the per-core SPMD program. Reduced sizes for dev testing."""
    nc = bacc.Bacc(
        "TRN2",
        target_bir_lowering=False,
        debug=False,
        enable_asserts=False,
        num_devices=N_CORES,
    )
    e_pc = n_etiles * 128
    h_full = n_htiles * 128
    xT = nc.dram_tensor("xT", [h_full, TOK_PC], F32R, kind="ExternalInput")
    gw = nc.dram_tensor("gw", [n_experts, h_full, e_pc], F32R, kind="ExternalInput")
    uw = nc.dram_tensor("uw", [n_experts, h_full, e_pc], F32R, kind="ExternalInput")
    dw = nc.dram_tensor("dw", [n_experts, e_pc, h_full], F32R, kind="ExternalInput")
    out = nc.dram_tensor("out", [n_experts, h_full, TOK_PC], F32, kind="ExternalOutput")

    silu = mybir.ActivationFunctionType.Silu
    hh_per_chunk = n_htiles // 2  # weight h-half chunks

    with tile.TileContext(nc) as tc:
        with (
            tc.tile_pool(name="xpool", bufs=8) as xpool,
            tc.tile_pool(name="gupool", bufs=3) as gupool,
            tc.tile_pool(name="dpool", bufs=3) as dpool,
            tc.tile_pool(name="actpool", bufs=n_etiles) as actpool,
            tc.tile_pool(name="silpool", bufs=2) as silpool,
            tc.tile_pool(name="opool", bufs=3) as opool,
            tc.tile_pool(name="gups", bufs=6, space="PSUM") as gups,
            tc.tile_pool(name="ops", bufs=2, space="PSUM") as ops,
        ):
            xch_tiles = 4  # h-tiles per x chunk
            n_xch = n_htiles // xch_tiles
            for blk in range(n_blocks):
                t0 = blk * TBLK
                xchunks = []

                def emit_x(xc):
                    xcht = xpool.tile(
                        [128, xch_tiles, TBLK], F32R, tag="x", name="xcht"
                    )
                    h0 = xc * xch_tiles * 128
                    xeng = nc.sync if xc % 2 == 0 else nc.scalar
                    xeng.dma_start(
                        out=xcht,
                        in_=xT[
                            h0 : h0 + xch_tiles * 128, t0 : t0 + TBLK
                        ].rearrange("(i p) t -> p i t", p=128),
                    )
                    xchunks.append(xcht)

                def emit_wch(wdram, k, j, m):
                    wch = gupool.tile(
                        [128, hh_per_chunk, 128], F32R, tag="gu", name="wch"
                    )
                    h0 = m * hh_per_chunk * 128
                    e0 = j * 128
                    weng = nc.sync if wdram is gw else nc.scalar
                    weng.dma_start(
                        out=wch,
                        in_=wdram[
                            k, h0 : h0 + hh_per_chunk * 128, e0 : e0 + 128
                        ].rearrange("(hh p) e -> p hh e", p=128),
                    )
                    return wch

                def xts_at(hi):
                    return xchunks[hi // xch_tiles][:, hi % xch_tiles, :]

                # interleave x-chunk loads with the first e-tile's weight
                # prefetch so the PE can start ~10us in and pace with arrivals
                emit_x(0)
                emit_x(1)
                pre = {"g": emit_wch(gw, 0, 0, 0), "u": emit_wch(uw, 0, 0, 0)}
                for xc in range(2, n_xch):
                    emit_x(xc)

                for k in range(n_experts):
                    act_tiles = []
                    for j in range(n_etiles):
                        psg = [
                            gups.tile([128, TSUB], F32, tag="gups", name=f"psg{s}")
                            for s in range(N_TSUB)
                        ]
                        psu = [
                            gups.tile([128, TSUB], F32, tag="gups", name=f"psu{s}")
                            for s in range(N_TSUB)
                        ]
                        for m in range(2):
                            for wdram, ps in ((gw, psg), (uw, psu)):
                                if k == 0 and j == 0 and m == 0:
                                    wch = pre["g" if wdram is gw else "u"]
                                else:
                                    wch = emit_wch(wdram, k, j, m)
                                for s in range(N_TSUB):
                                    for hh in range(hh_per_chunk):
                                        hi = m * hh_per_chunk + hh
                                        nc.tensor.matmul(
                                            ps[s],
                                            wch[:, hh, :],
                                            xts_at(hi)[
                                                :, s * TSUB : (s + 1) * TSUB
                                            ],
                                            start=(hi == 0),
                                            stop=(hi == n_htiles - 1),
                                        )
                        act_j = actpool.tile([128, TBLK], F32R, tag="act", name="act_j")
                        for s in range(N_TSUB):
                            sil = silpool.tile([128, TSUB], F32, tag="sil", name="sil")
                            nc.scalar.activation(sil, psg[s], silu)
                            nc.vector.tensor_mul(
                                act_j[:, s * TSUB : (s + 1) * TSUB], sil, psu[s]
                            )
                        act_tiles.append(act_j)
                    for i in range(n_htiles):
                        dch = dpool.tile([128, n_etiles, 128], F32R, tag="d", name="dch")
                        nc.sync.dma_start(
                            out=dch,
                            in_=dw[k, :, i * 128 : (i + 1) * 128].rearrange(
                                "(j p) h -> p j h", p=128
                            ),
                        )
                        pso = [
                            ops.tile([128, TSUB], F32, tag="ops", name=f"pso{s}")
                            for s in range(N_TSUB)
                        ]
                        for s in range(N_TSUB):
                            for j in range(n_etiles):
                                nc.tensor.matmul(
                                    pso[s],
                                    dch[:, j, :],
                                    act_tiles[j][:, s * TSUB : (s + 1) * TSUB],
                                    start=(j == 0),
                                    stop=(j == n_etiles - 1),
                                )
                        ot = opool.tile([128, TBLK], F32, tag="ot", name="ot")
                        for s in range(N_TSUB):
                            nc.vector.tensor_copy(
                                ot[:, s * TSUB : (s + 1) * TSUB], pso[s]
                            )
                        nc.gpsimd.dma_start(
                            out=out[k, i * 128 : (i + 1) * 128, t0 : t0 + TBLK],
                            in_=ot,
                        )
    nc.compile()
    return nc


@functools.cache
def _built_full():
    return _build()


def kernel(x, gate_proj, up_proj, down_proj, expert_idx):
    x = np.asarray(x)
    idx = np.asarray(expert_idx)
    gate = np.asarray(gate_proj)[idx]  # [K, H, E]
    up = np.asarray(up_proj)[idx]
    down = np.asarray(down_proj)[idx]  # [K, E, H]

    nc = _built_full()

    xf = x.reshape(TOK, H)
    in_maps = []
    for c in range(N_CORES):
        tau, eps = divmod(c, 2)
        tsl = slice(TOK_PC * tau, TOK_PC * (tau + 1))
        esl = slice(E_PC * eps, E_PC * (eps + 1))
        in_maps.append(
            {
                "xT": np.ascontiguousarray(xf[tsl].T),
                "gw": np.ascontiguousarray(gate[:, :, esl]),
                "uw": np.ascontiguousarray(up[:, :, esl]),
                "dw": np.ascontiguousarray(down[:, esl, :]),
            }
        )

    res = run_bass_kernel_spmd(nc, in_maps, core_ids=list(range(N_CORES)))

    out = np.empty((TOK, K, H), dtype=np.float32)
    for tau in range(4):
        part = res.results[2 * tau]["out"] + res.results[2 * tau + 1]["out"]
        # part: [K, H, TOK_PC] -> [TOK_PC, K, H]
        out[TOK_PC * tau : TOK_PC * (tau + 1)] = part.transpose(2, 0, 1)
    return out.reshape(B, S, K, H)
